# revision 88
# baseline (speedup 1.0000x reference)
"""Trainium2 Bass kernel for nn_MultiHeadAttention (B=4, S=2048, D=512, H=8, DH=64).

Sharding: 8 cores = 4 batches x 2 query-halves. Each core computes full
attention for all 8 heads over its 1024 query rows (K/V projections are
duplicated within a batch pair). The output is a pure concatenation.

Per-core pipeline (bf16 datapath, fp32 PSUM accumulation):
  1. Inputs/weights are pre-cast to bf16 AND pre-transposed on the host;
     X^T (feature-major) loads are plain contiguous DMAs spread over the
     sync + scalar queues (on-device DMA_TRANSPOSE costs ~2.4us of
     descriptor-generation per chunk and serialized the whole prologue).
  2. Dummy matmuls on a memset tile warm the PE clock (HAM) to K=8/8
     during the DMA prologue.
  3. Project: Q^T, K^T feature-major ([512, S]); V natural ([S, 512]) with
     an extra all-ones column appended per head (65-col layout).
  4. Attention per head, per 128-row k-block:
       S^T[k, q] = K^T_h(stationary) @ Q^T_h   (contraction = d_head 64)
       P^T = exp(S^T / 8)                       (ScalarE, PSUM -> SBUF bf16)
       z^T[65, q] += [V_h | 1](stationary) @ P^T  (row 64 = softmax denom)
  5. Normalize: reciprocal of row 64, broadcast, multiply -> Z^T.
  6. Output projection from Z^T + bias in fp32, DMA out.
"""

import os
import sys

import numpy as np

sys.path.insert(0, "/opt/trn_rl_repo")

import ml_dtypes
import concourse.bacc as bacc
import concourse.bass as bass
import concourse.mybir as mybir
import concourse.tile as tile
from concourse import bass_utils

F32 = mybir.dt.float32
BF16 = mybir.dt.bfloat16

B, S, D, H, DH = 4, 2048, 512, 8, 64
SQ = S // 2          # query rows per core
NKB = S // 128       # 16 k-blocks
NFT = D // 128       # 4 feature tiles
NQB = SQ // 128      # 8 query blocks
N_CORES = 8

Exp = mybir.ActivationFunctionType.Exp
Identity = mybir.ActivationFunctionType.Identity


def build_program(dbg=False):
    nc = bacc.Bacc("TRN2", target_bir_lowering=False, debug=False)

    xq = nc.dram_tensor("XQT", [D, SQ], BF16, kind="ExternalInput").ap()
    xk = nc.dram_tensor("XKT", [D, S], BF16, kind="ExternalInput").ap()
    xv = nc.dram_tensor("XVT", [D, S], BF16, kind="ExternalInput").ap()
    wq = nc.dram_tensor("Wq", [D, D], BF16, kind="ExternalInput").ap()
    wk = nc.dram_tensor("Wk", [D, D], BF16, kind="ExternalInput").ap()
    wv = nc.dram_tensor("Wv", [D, D], BF16, kind="ExternalInput").ap()
    wo = nc.dram_tensor("Wo", [D, D], BF16, kind="ExternalInput").ap()
    bq = nc.dram_tensor("bq", [D, 1], F32, kind="ExternalInput").ap()
    bk = nc.dram_tensor("bk", [D, 1], F32, kind="ExternalInput").ap()
    bv = nc.dram_tensor("bv", [1, D], F32, kind="ExternalInput").ap()
    bo = nc.dram_tensor("bo", [1, D], F32, kind="ExternalInput").ap()
    out = nc.dram_tensor("OUT", [SQ, D], F32, kind="ExternalOutput").ap()

    from contextlib import ExitStack

    with tile.TileContext(nc) as tc, ExitStack() as ctx:
        const = ctx.enter_context(tc.tile_pool(name="const", bufs=1))
        xt_pool = ctx.enter_context(tc.tile_pool(name="xt", bufs=1))
        w_pool = ctx.enter_context(tc.tile_pool(name="w", bufs=1))
        kt_pool = ctx.enter_context(tc.tile_pool(name="kt", bufs=1))
        qt_pool = ctx.enter_context(tc.tile_pool(name="qt", bufs=1))
        v_pool = ctx.enter_context(tc.tile_pool(name="v", bufs=1))
        # 27 exp-slab buffers: each pair produces 32 slabs before the next
        # pair's z-drain consumes them, so every buffer beyond 23 directly
        # shortens the exp WAR stall at pair boundaries
        p_pool = ctx.enter_context(tc.tile_pool(name="p", bufs=33))
        zt_pool = ctx.enter_context(tc.tile_pool(name="zt", bufs=1))
        nrm_pool = ctx.enter_context(tc.tile_pool(name="nrm", bufs=1))
        out_pool = ctx.enter_context(tc.tile_pool(name="outp", bufs=3))

        # One PSUM pool, 4 tags x 2 banks = all 8 banks.  Prefix (projection)
        # and epilogue tiles rotate through the same tags that attention uses
        # for sA/sB/zA/zB.
        ps = ctx.enter_context(tc.tile_pool(name="ps", bufs=1, space="PSUM"))
        ps_ctr = [0, 0]

        def ps_tile(shape, tags, name):
            i = 0 if tags == "ab" else 1
            tag = ("a", "b", "c", "d")[2 * i + ps_ctr[i] % 2]
            ps_ctr[i] += 1
            return ps.tile(shape, F32, tag=tag, name=name, padded_shape=[128, SQ])

        # warm the ScalarE Exp table first thing (the first ACTIVATE
        # otherwise pays the ~2.7us ACT_TABLE_LOAD inline)
        warm = nrm_pool.tile([1, 8], F32, tag="warm")
        nc.gpsimd.memset(warm[:], 0.0)
        warm2 = nrm_pool.tile([1, 8], F32, tag="warm2")
        nc.scalar.activation(warm2[:], warm[:], Exp, scale=0.125)

        # PE warm-up: dummy matmuls on a memset tile so HAM reaches K=8/8
        # before the first real matmul (the DMA prologue would otherwise
        # leave the PE at half clock for its first ~3.4us of work)
        scratch = const.tile([128, 256], BF16, tag="scratch")
        nc.gpsimd.memset(scratch[:], 0.0)
        dummy_ps = ps.tile([128, 256], F32, tag="a", name="dummy",
                           padded_shape=[128, SQ])
        for _ in range(48):
            nc.tensor.matmul(dummy_ps[:], scratch[:, 0:128], scratch[:],
                             start=True, stop=True)

        # ---- weights: one DMA per tensor, sliced into 4 contraction chunks ----
        def load_w(wdram, name, eng):
            big = w_pool.tile([128, NFT * D], BF16, tag=f"w{name}", name=f"w{name}")
            eng.dma_start(
                big[:].rearrange("p (g c) -> p g c", g=NFT),
                wdram.rearrange("(g p) c -> p g c", p=128),
            )
            return [big[:, D * mc:D * (mc + 1)] for mc in range(NFT)]

        # ---- X^T loads: plain contiguous DMAs (host pre-transposed X) ----
        def load_xt(xdram, nrows, name, engs):
            tiles = []
            for ft in range(NFT):
                t = xt_pool.tile([128, nrows], BF16, tag=f"xt{name}{ft}",
                                 name=f"xt{name}{ft}", padded_shape=[128, S])
                engs[ft % len(engs)].dma_start(
                    t[:], xdram[128 * ft:128 * (ft + 1), :]
                )
                tiles.append(t)
            return tiles

        # ---- DMA loads: priority order.  The first exp needs wk + K-cols
        # 0-1023 + wq + all of Q; those go first, with the X chunks split
        # across the scalar/gpsimd queues so transfers overlap the weight
        # loads running on sync. ----
        def xt_tiles(name, nrows):
            # padded to the actual row count (padding xq to [128,S] wasted
            # 8KB/partition that the p pool needs for exp slabs)
            return [xt_pool.tile([128, nrows], BF16, tag=f"xt{name}{ft}",
                                 name=f"xt{name}{ft}", padded_shape=[128, nrows])
                    for ft in range(NFT)]

        xkt = xt_tiles("k", S)
        xqt = xt_tiles("q", SQ)
        xvt = xt_tiles("v", S)

        # balance the 3MB of first-exp-critical bytes across the two HWDGE
        # queues (sync ~1.5MB, scalar ~1.5MB at ~130GB/s each)
        wk_t = load_w(wk, "k", nc.sync)
        wq_t = load_w(wq, "q", nc.scalar)
        for ft in range(NFT):           # K-blocks 0-7 (needed by s_exp(0,0))
            (nc.sync if ft < 2 else nc.scalar).dma_start(
                xkt[ft][:, 0:1024], xk[128 * ft:128 * (ft + 1), 0:1024])
        for ft in range(NFT):           # all of Q
            (nc.sync if ft < 2 else nc.scalar).dma_start(
                xqt[ft][:], xq[128 * ft:128 * (ft + 1), :])
        for ft in range(NFT):           # K-blocks 8-15 (needed from slot 8)
            (nc.sync if ft < 2 else nc.scalar).dma_start(
                xkt[ft][:, 1024:2048], xk[128 * ft:128 * (ft + 1), 1024:2048])
        bq_all = const.tile([128, NFT], F32, tag="bqa")
        nc.sync.dma_start(
            bq_all[:].rearrange("p (g o) -> p g o", g=NFT),
            bq.rearrange("(g p) o -> p g o", p=128),
        )
        bk_all = const.tile([128, NFT], F32, tag="bka")
        nc.sync.dma_start(
            bk_all[:].rearrange("p (g o) -> p g o", g=NFT),
            bk.rearrange("(g p) o -> p g o", p=128),
        )
        bq_t = [bq_all[:, ft:ft + 1] for ft in range(NFT)]
        bk_t = [bk_all[:, ft:ft + 1] for ft in range(NFT)]
        bv_row = const.tile([1, D], F32, tag="bvr")
        nc.sync.dma_start(bv_row[:], bv[:])
        bv_bc = const.tile([128, D], F32, tag="bvb")
        nc.gpsimd.partition_broadcast(bv_bc[:], bv_row[:], channels=128)
        bo_row = const.tile([1, D], F32, tag="bor")
        nc.sync.dma_start(bo_row[:], bo[:])
        bo_rb = const.tile([1, D], BF16, tag="borb")
        nc.vector.tensor_copy(bo_rb[:], bo_row[:])
        ones_row = const.tile([1, 128], BF16, tag="ones")
        nc.gpsimd.memset(ones_row[:], 1.0)

        wv_t = load_w(wv, "v", nc.sync)
        for hh in range(2):             # V halves, k-blocks 0-7 first
            for ft in range(NFT):
                (nc.sync if ft % 2 else nc.scalar).dma_start(
                    xvt[ft][:, 1024 * hh:1024 * (hh + 1)],
                    xv[128 * ft:128 * (ft + 1), 1024 * hh:1024 * (hh + 1)])
        wo_t = load_w(wo, "o", nc.sync)

        k_t = [kt_pool.tile([128, S], BF16, tag=f"kt{ft}", name=f"kt{ft}")
               for ft in range(NFT)]
        q_t = [qt_pool.tile([128, SQ], BF16, tag=f"qt{ft}", name=f"qt{ft}")
               for ft in range(NFT)]

        def proj_k_chunk(ft, sc):
            pj = ps_tile([128, 1024], "cd", f"pjk{ft}{sc}")
            for h2 in range(2):
                for mc in range(NFT):
                    nc.tensor.matmul(
                        pj[:, 512 * h2:512 * (h2 + 1)],
                        wk_t[mc][:, 128 * ft:128 * (ft + 1)],
                        xkt[mc][:, 1024 * sc + 512 * h2:1024 * sc + 512 * (h2 + 1)],
                        start=(mc == 0),
                        stop=(mc == NFT - 1),
                    )
            nc.any.tensor_scalar_add(
                k_t[ft][:, 1024 * sc:1024 * (sc + 1)], pj[:], bk_t[ft][:],
            )

        def proj_q_chunk(ft):
            pj = ps_tile([128, 1024], "cd", f"pjq{ft}")
            for h2 in range(2):
                for mc in range(NFT):
                    nc.tensor.matmul(
                        pj[:, 512 * h2:512 * (h2 + 1)],
                        wq_t[mc][:, 128 * ft:128 * (ft + 1)],
                        xqt[mc][:, 512 * h2:512 * (h2 + 1)],
                        start=(mc == 0),
                        stop=(mc == NFT - 1),
                    )
            nc.any.tensor_scalar_add(q_t[ft][:], pj[:], bq_t[ft][:])

        def proj_kq(ft):
            proj_k_chunk(ft, 0)
            proj_q_chunk(ft)
            proj_k_chunk(ft, 1)

        # ---- slot-scheduled emission ----------------------------------
        # PE is the binding engine; emit its work as one interleaved stream:
        #  - S + exp for (pair, kb) runs in slot (pair, kb)
        #  - V projections ride in pair-0 slots (PSUM c/d tags)
        #  - each pair's z-accumulation is deferred while c/d is busy, then
        #    drains two-groups-per-slot once its zA/zB tiles pin c/d
        #  - K/Q projections for pair p+1 slot into the c/d window between
        #    norm(p-1) and z(p) pinning
        proj_k_chunk(0, 0)
        proj_q_chunk(0)

        VW = H * (DH + 1)  # 520: per head 64 value cols + 1 ones col
        v_aug = [v_pool.tile([128, VW], BF16, tag=f"v{kb}", name=f"v{kb}")
                 for kb in range(NKB)]

        def v_group(kb):
            nc.gpsimd.memset(
                v_aug[kb][:].rearrange("p (h c) -> p h c", h=H)[:, :, DH:DH + 1],
                1.0,
            )
            pj = ps_tile([128, 512], "cd", f"pjv{kb}")
            for mc in range(NFT):
                nc.tensor.matmul(
                    pj[:],
                    xvt[mc][:, 128 * kb:128 * (kb + 1)],
                    wv_t[mc][:],
                    start=(mc == 0),
                    stop=(mc == NFT - 1),
                )
            nc.any.tensor_add(
                v_aug[kb][:].rearrange("p (h c) -> p h c", h=H)[:, :, 0:DH],
                pj[:].rearrange("p (h c) -> p h c", h=H),
                bv_bc[:].rearrange("p (h c) -> p h c", h=H),
            )

        z_t = [zt_pool.tile([128, SQ], BF16, tag=f"zt{p}", name=f"zt{p}")
               for p in range(NFT)]
        po_sb = [zt_pool.tile([128, D], BF16, tag=f"po{qb}", name=f"po{qb}")
                 for qb in range(NQB)]
        p_slabs = {}
        z_tiles = {}

        def s_exp(pair, kb):
            sA = ps.tile([128, SQ], F32, tag="a", name=f"sA{pair}_{kb}")
            sB = ps.tile([128, SQ], F32, tag="b", name=f"sB{pair}_{kb}")
            for qc in range(SQ // 512):
                qs = slice(512 * qc, 512 * (qc + 1))
                nc.tensor.matmul(
                    sA[:, qs],
                    k_t[pair][0:DH, 128 * kb:128 * (kb + 1)],
                    q_t[pair][0:DH, qs],
                    start=True, stop=True,
                    tile_position=(0, 0),
                )
                nc.tensor.matmul(
                    sB[:, qs],
                    k_t[pair][DH:128, 128 * kb:128 * (kb + 1)],
                    q_t[pair][DH:128, qs],
                    start=True, stop=True,
                    tile_position=(64, 0),
                )
            pA = p_pool.tile([128, SQ], BF16, tag="p", name=f"pA{pair}_{kb}")
            nc.scalar.activation(pA[:], sA[:], Exp, scale=0.125)
            pB = p_pool.tile([128, SQ], BF16, tag="p", name=f"pB{pair}_{kb}")
            nc.scalar.activation(pB[:], sB[:], Exp, scale=0.125)
            p_slabs[(pair, kb)] = (pA, pB)

        def z_alloc(pair):
            zA = ps.tile([DH + 1, SQ], F32, tag="c", name=f"zA{pair}",
                         padded_shape=[128, SQ])
            zB = ps.tile([DH + 1, SQ], F32, tag="d", name=f"zB{pair}",
                         padded_shape=[128, SQ])
            z_tiles[pair] = (zA, zB)

        def z_group(pair, kb):
            zA, zB = z_tiles[pair]
            pA, pB = p_slabs.pop((pair, kb))
            hA, hB = 2 * pair, 2 * pair + 1
            for qc in range(SQ // 512):
                qs = slice(512 * qc, 512 * (qc + 1))
                nc.tensor.matmul(
                    zA[:, qs],
                    v_aug[kb][:, 65 * hA:65 * hA + 65],
                    pA[:, qs],
                    start=(kb == 0), stop=(kb == NKB - 1),
                    skip_group_check=True,
                )
                nc.tensor.matmul(
                    zB[:, qs],
                    v_aug[kb][:, 65 * hB:65 * hB + 65],
                    pB[:, qs],
                    start=(kb == 0), stop=(kb == NKB - 1),
                    skip_group_check=True,
                )

        def norm(pair):
            # Head A's raw z is drained to SBUF immediately (3 DVE reads
            # free PSUM tag c ~4us earlier for the next projections) and
            # normalized in place later; head B keeps the direct PSUM-read
            # multiply (all DVE input operands must stay at partition base
            # 0, so an in-place z_t[64:128] multiply is not expressible).
            zA, zB = z_tiles.pop(pair)
            rowcA = nrm_pool.tile([1, SQ], F32, tag="rowc")
            nc.vector.tensor_copy(rowcA[:], zA[DH:DH + 1, :])
            nc.vector.tensor_copy(z_t[pair][0:DH, :], zA[0:DH, :])
            rowcB = nrm_pool.tile([1, SQ], F32, tag="rowc")
            nc.vector.tensor_copy(rowcB[:], zB[DH:DH + 1, :])
            for rowc, half, src in ((rowcA, 0, None), (rowcB, 1, zB)):
                recip = nrm_pool.tile([1, SQ], F32, tag="recip")
                nc.vector.reciprocal_approx_fast(recip[:], rowc[:])
                rbc = nrm_pool.tile([DH, SQ], F32, tag="rbc")
                nc.gpsimd.partition_broadcast(rbc[:], recip[:], channels=DH)
                nc.vector.tensor_mul(
                    z_t[pair][64 * half:64 * half + 64, :],
                    z_t[pair][0:DH, :] if src is None else src[0:DH, :],
                    rbc[:]
                )

        # slot schedule: slot (p, kb) -> extra emissions after S+exp
        feeder = [lambda: proj_k_chunk(0, 1),
                  lambda: proj_k_chunk(1, 0),
                  lambda: proj_q_chunk(1),
                  lambda: proj_k_chunk(1, 1)]
        feeder += [(lambda k: (lambda: v_group(k)))(kb) for kb in range(NKB)]
        fi = [0]

        def feed(n):
            for _ in range(n):
                if fi[0] < len(feeder):
                    feeder[fi[0]]()
                    fi[0] += 1

        for pair in range(NFT):
            for kb in range(NKB):
                s_exp(pair, kb)
                if pair == 0:
                    feed(2 if kb < 4 else 1)
                    if kb == NKB - 1:
                        feed(len(feeder))
                elif pair == 1:
                    if kb == 0:
                        z_alloc(0)
                    if kb < 8:
                        z_group(0, 2 * kb)
                        z_group(0, 2 * kb + 1)
                    elif kb == 8:
                        norm(0)
                    elif kb == 9:
                        proj_k_chunk(2, 0)
                    elif kb == 10:
                        proj_q_chunk(2)
                    elif kb == 11:
                        proj_k_chunk(2, 1)
                    elif kb == 12:
                        z_alloc(1)
                    if kb >= 12:
                        z_group(1, 2 * (kb - 12))
                        z_group(1, 2 * (kb - 12) + 1)
                elif pair == 2:
                    if kb < 4:
                        z_group(1, 8 + 2 * kb)
                        z_group(1, 8 + 2 * kb + 1)
                    elif kb == 4:
                        norm(1)
                    elif kb == 5:
                        proj_k_chunk(3, 0)
                    elif kb == 6:
                        proj_q_chunk(3)
                    elif kb == 7:
                        proj_k_chunk(3, 1)
                    elif kb == 8:
                        z_alloc(2)
                    if kb >= 8:
                        z_group(2, 2 * (kb - 8))
                        z_group(2, 2 * (kb - 8) + 1)
                else:
                    if kb == 0:
                        norm(2)
                    elif kb == 1:
                        z_alloc(3)
                    if kb >= 1:
                        z_group(3, kb - 1)
            if pair == NFT - 1:
                z_group(3, 15)
                # norm(3) owns DVE/GpSimd; out-proj pass 1 (pairs 0-2 plus
                # the bias as a rank-1 matmul) runs on the PE with the freed
                # a/b banks and drains via the now-idle ScalarE
                norm(3)
                for qb in range(NQB):
                    po = ps_tile([128, D], "ab", f"po1{qb}")
                    for p4 in range(3):
                        nc.tensor.matmul(
                            po[:],
                            z_t[p4][:, 128 * qb:128 * (qb + 1)],
                            wo_t[p4][:],
                            start=(p4 == 0),
                            stop=False,
                        )
                    nc.tensor.matmul(po[:], ones_row[:], bo_rb[:],
                                     start=False, stop=True)
                    nc.scalar.activation(po_sb[qb][:], po[:],
                                         mybir.ActivationFunctionType.Copy)

        # ---- output projection pass 2: add pair 3, DMA out (scalar queue
        # is busy with the pass-1 drains, so DMA via sync/gpsimd) ----
        out_engs = [nc.sync, nc.gpsimd]
        for qb in range(NQB):
            po = ps_tile([128, D], "ab", f"po2{qb}")
            nc.tensor.matmul(
                po[:],
                z_t[3][:, 128 * qb:128 * (qb + 1)],
                wo_t[3][:],
                start=True, stop=True,
            )
            ot = out_pool.tile([128, D], F32, tag="ot")
            nc.any.tensor_add(ot[:], po[:], po_sb[qb][:])
            out_engs[qb % 2].dma_start(out[128 * qb:128 * (qb + 1), :], ot[:])

    nc.compile()
    return nc


_NC = None
LAST_RESULTS = None


def _get_nc():
    global _NC
    if _NC is None:
        _NC = build_program(dbg=bool(int(os.environ.get("KERNEL_DEBUG", "0"))))
    return _NC


def _bf(x):
    return np.ascontiguousarray(np.asarray(x).astype(ml_dtypes.bfloat16))


def kernel(Q, K, V, Wq, bq, Wk, bk, Wv, bv, Wo, bo):
    global LAST_RESULTS
    nc = _get_nc()
    Qb, Kb, Vb = _bf(Q), _bf(K), _bf(V)
    shared = {
        "Wq": _bf(Wq),
        "Wk": _bf(Wk),
        "Wv": _bf(Wv),
        "Wo": _bf(Wo),
        "bq": np.ascontiguousarray(np.asarray(bq, np.float32).reshape(D, 1)),
        "bk": np.ascontiguousarray(np.asarray(bk, np.float32).reshape(D, 1)),
        "bv": np.ascontiguousarray(np.asarray(bv, np.float32).reshape(1, D)),
        "bo": np.ascontiguousarray(np.asarray(bo, np.float32).reshape(1, D)),
    }
    kt = [np.ascontiguousarray(Kb[b].T) for b in range(B)]
    vt = [np.ascontiguousarray(Vb[b].T) for b in range(B)]
    in_maps = []
    for c in range(N_CORES):
        b, qh = c // 2, c % 2
        in_maps.append({
            "XQT": np.ascontiguousarray(Qb[b, SQ * qh:SQ * (qh + 1)].T),
            "XKT": kt[b],
            "XVT": vt[b],
            **shared,
        })
    trace = bool(int(os.environ.get("KERNEL_TRACE", "0")))
    res = bass_utils.run_bass_kernel_spmd(
        nc, in_maps, core_ids=list(range(N_CORES)), trace=trace,
    )
    LAST_RESULTS = res
    out = np.empty((B, S, D), dtype=np.float32)
    for c in range(N_CORES):
        b, qh = c // 2, c % 2
        out[b, SQ * qh:SQ * (qh + 1)] = res.results[c]["OUT"]
    return out


# revision 90
# speedup vs baseline: 1.0070x; 1.0070x over previous
"""Trainium2 Bass kernel for nn_MultiHeadAttention (B=4, S=2048, D=512, H=8, DH=64).

Sharding: 8 cores = 4 batches x 2 query-halves. Each core computes full
attention for all 8 heads over its 1024 query rows (K/V projections are
duplicated within a batch pair). The output is a pure concatenation.

Per-core pipeline (bf16 datapath, fp32 PSUM accumulation):
  1. Inputs/weights are pre-cast to bf16 AND pre-transposed on the host;
     X^T (feature-major) loads are plain contiguous DMAs spread over the
     sync + scalar queues (on-device DMA_TRANSPOSE costs ~2.4us of
     descriptor-generation per chunk and serialized the whole prologue).
  2. Dummy matmuls on a memset tile warm the PE clock (HAM) to K=8/8
     during the DMA prologue.
  3. Project: Q^T, K^T feature-major ([512, S]); V natural ([S, 512]) with
     an extra all-ones column appended per head (65-col layout).
  4. Attention per head, per 128-row k-block:
       S^T[k, q] = K^T_h(stationary) @ Q^T_h   (contraction = d_head 64)
       P^T = exp(S^T / 8)                       (ScalarE, PSUM -> SBUF bf16)
       z^T[65, q] += [V_h | 1](stationary) @ P^T  (row 64 = softmax denom)
  5. Normalize: reciprocal of row 64, broadcast, multiply -> Z^T.
  6. Output projection from Z^T + bias in fp32, DMA out.
"""

import os
import sys

import numpy as np

sys.path.insert(0, "/opt/trn_rl_repo")

import ml_dtypes
import concourse.bacc as bacc
import concourse.bass as bass
import concourse.mybir as mybir
import concourse.tile as tile
from concourse import bass_utils

F32 = mybir.dt.float32
BF16 = mybir.dt.bfloat16

B, S, D, H, DH = 4, 2048, 512, 8, 64
SQ = S // 2          # query rows per core
NKB = S // 128       # 16 k-blocks
NFT = D // 128       # 4 feature tiles
NQB = SQ // 128      # 8 query blocks
N_CORES = 8

Exp = mybir.ActivationFunctionType.Exp
Identity = mybir.ActivationFunctionType.Identity


def build_program(dbg=False):
    nc = bacc.Bacc("TRN2", target_bir_lowering=False, debug=False)

    xq = nc.dram_tensor("XQT", [D, SQ], BF16, kind="ExternalInput").ap()
    xk = nc.dram_tensor("XKT", [D, S], BF16, kind="ExternalInput").ap()
    xv = nc.dram_tensor("XVT", [D, S], BF16, kind="ExternalInput").ap()
    wq = nc.dram_tensor("Wq", [D, D], BF16, kind="ExternalInput").ap()
    wk = nc.dram_tensor("Wk", [D, D], BF16, kind="ExternalInput").ap()
    wv = nc.dram_tensor("Wv", [D, D], BF16, kind="ExternalInput").ap()
    wo = nc.dram_tensor("Wo", [D, D], BF16, kind="ExternalInput").ap()
    bq = nc.dram_tensor("bq", [D, 1], F32, kind="ExternalInput").ap()
    bk = nc.dram_tensor("bk", [D, 1], F32, kind="ExternalInput").ap()
    bv = nc.dram_tensor("bv", [1, D], F32, kind="ExternalInput").ap()
    bo = nc.dram_tensor("bo", [1, D], F32, kind="ExternalInput").ap()
    out = nc.dram_tensor("OUT", [SQ, D], F32, kind="ExternalOutput").ap()

    from contextlib import ExitStack

    with tile.TileContext(nc) as tc, ExitStack() as ctx:
        const = ctx.enter_context(tc.tile_pool(name="const", bufs=1))
        xt_pool = ctx.enter_context(tc.tile_pool(name="xt", bufs=1))
        w_pool = ctx.enter_context(tc.tile_pool(name="w", bufs=1))
        kt_pool = ctx.enter_context(tc.tile_pool(name="kt", bufs=1))
        qt_pool = ctx.enter_context(tc.tile_pool(name="qt", bufs=1))
        v_pool = ctx.enter_context(tc.tile_pool(name="v", bufs=1))
        # 27 exp-slab buffers: each pair produces 32 slabs before the next
        # pair's z-drain consumes them, so every buffer beyond 23 directly
        # shortens the exp WAR stall at pair boundaries
        p_pool = ctx.enter_context(tc.tile_pool(name="p", bufs=34))
        zt_pool = ctx.enter_context(tc.tile_pool(name="zt", bufs=1))
        nrm_pool = ctx.enter_context(tc.tile_pool(name="nrm", bufs=1))
        out_pool = ctx.enter_context(tc.tile_pool(name="outp", bufs=2))

        # One PSUM pool, 4 tags x 2 banks = all 8 banks.  Prefix (projection)
        # and epilogue tiles rotate through the same tags that attention uses
        # for sA/sB/zA/zB.
        ps = ctx.enter_context(tc.tile_pool(name="ps", bufs=1, space="PSUM"))
        ps_ctr = [0, 0]

        def ps_tile(shape, tags, name):
            i = 0 if tags == "ab" else 1
            tag = ("a", "b", "c", "d")[2 * i + ps_ctr[i] % 2]
            ps_ctr[i] += 1
            return ps.tile(shape, F32, tag=tag, name=name, padded_shape=[128, SQ])

        # warm the ScalarE Exp table first thing (the first ACTIVATE
        # otherwise pays the ~2.7us ACT_TABLE_LOAD inline)
        warm = nrm_pool.tile([1, 8], F32, tag="warm")
        nc.gpsimd.memset(warm[:], 0.0)
        warm2 = nrm_pool.tile([1, 8], F32, tag="warm2")
        nc.scalar.activation(warm2[:], warm[:], Exp, scale=0.125)

        # PE warm-up: dummy matmuls on a memset tile so HAM reaches K=8/8
        # before the first real matmul (the DMA prologue would otherwise
        # leave the PE at half clock for its first ~3.4us of work)
        scratch = const.tile([128, 256], BF16, tag="scratch")
        nc.gpsimd.memset(scratch[:], 0.0)
        dummy_ps = ps.tile([128, 256], F32, tag="a", name="dummy",
                           padded_shape=[128, SQ])
        for _ in range(48):
            nc.tensor.matmul(dummy_ps[:], scratch[:, 0:128], scratch[:],
                             start=True, stop=True)

        # ---- weights: one DMA per tensor, sliced into 4 contraction chunks ----
        def load_w(wdram, name, eng):
            big = w_pool.tile([128, NFT * D], BF16, tag=f"w{name}", name=f"w{name}")
            eng.dma_start(
                big[:].rearrange("p (g c) -> p g c", g=NFT),
                wdram.rearrange("(g p) c -> p g c", p=128),
            )
            return [big[:, D * mc:D * (mc + 1)] for mc in range(NFT)]

        # ---- X^T loads: plain contiguous DMAs (host pre-transposed X) ----
        def load_xt(xdram, nrows, name, engs):
            tiles = []
            for ft in range(NFT):
                t = xt_pool.tile([128, nrows], BF16, tag=f"xt{name}{ft}",
                                 name=f"xt{name}{ft}", padded_shape=[128, S])
                engs[ft % len(engs)].dma_start(
                    t[:], xdram[128 * ft:128 * (ft + 1), :]
                )
                tiles.append(t)
            return tiles

        # ---- DMA loads: priority order.  The first exp needs wk + K-cols
        # 0-1023 + wq + all of Q; those go first, with the X chunks split
        # across the scalar/gpsimd queues so transfers overlap the weight
        # loads running on sync. ----
        def xt_tiles(name, nrows):
            # padded to the actual row count (padding xq to [128,S] wasted
            # 8KB/partition that the p pool needs for exp slabs)
            return [xt_pool.tile([128, nrows], BF16, tag=f"xt{name}{ft}",
                                 name=f"xt{name}{ft}", padded_shape=[128, nrows])
                    for ft in range(NFT)]

        xkt = xt_tiles("k", S)
        xqt = xt_tiles("q", SQ)
        xvt = xt_tiles("v", S)

        # balance the 3MB of first-exp-critical bytes across the two HWDGE
        # queues (sync ~1.5MB, scalar ~1.5MB at ~130GB/s each)
        wk_t = load_w(wk, "k", nc.sync)
        wq_t = load_w(wq, "q", nc.scalar)
        for ft in range(NFT):           # K-blocks 0-7 (needed by s_exp(0,0))
            (nc.sync if ft < 2 else nc.scalar).dma_start(
                xkt[ft][:, 0:1024], xk[128 * ft:128 * (ft + 1), 0:1024])
        for ft in range(NFT):           # all of Q
            (nc.sync if ft < 2 else nc.scalar).dma_start(
                xqt[ft][:], xq[128 * ft:128 * (ft + 1), :])
        for ft in range(NFT):           # K-blocks 8-15 (needed from slot 8)
            (nc.sync if ft < 2 else nc.scalar).dma_start(
                xkt[ft][:, 1024:2048], xk[128 * ft:128 * (ft + 1), 1024:2048])
        bq_all = const.tile([128, NFT], F32, tag="bqa")
        nc.sync.dma_start(
            bq_all[:].rearrange("p (g o) -> p g o", g=NFT),
            bq.rearrange("(g p) o -> p g o", p=128),
        )
        bk_all = const.tile([128, NFT], F32, tag="bka")
        nc.sync.dma_start(
            bk_all[:].rearrange("p (g o) -> p g o", g=NFT),
            bk.rearrange("(g p) o -> p g o", p=128),
        )
        bq_t = [bq_all[:, ft:ft + 1] for ft in range(NFT)]
        bk_t = [bk_all[:, ft:ft + 1] for ft in range(NFT)]
        bv_row = const.tile([1, D], F32, tag="bvr")
        nc.sync.dma_start(bv_row[:], bv[:])
        bv_bc = const.tile([128, D], F32, tag="bvb")
        nc.gpsimd.partition_broadcast(bv_bc[:], bv_row[:], channels=128)
        bo_row = const.tile([1, D], F32, tag="bor")
        nc.sync.dma_start(bo_row[:], bo[:])
        bo_rb = const.tile([1, D], BF16, tag="borb")
        nc.vector.tensor_copy(bo_rb[:], bo_row[:])
        ones_row = const.tile([1, 128], BF16, tag="ones")
        nc.gpsimd.memset(ones_row[:], 1.0)

        wv_t = load_w(wv, "v", nc.sync)
        for hh in range(2):             # V halves, k-blocks 0-7 first
            for ft in range(NFT):
                (nc.sync if ft % 2 else nc.scalar).dma_start(
                    xvt[ft][:, 1024 * hh:1024 * (hh + 1)],
                    xv[128 * ft:128 * (ft + 1), 1024 * hh:1024 * (hh + 1)])
        wo_t = load_w(wo, "o", nc.sync)

        k_t = [kt_pool.tile([128, S], BF16, tag=f"kt{ft}", name=f"kt{ft}")
               for ft in range(NFT)]
        q_t = [qt_pool.tile([128, SQ], BF16, tag=f"qt{ft}", name=f"qt{ft}")
               for ft in range(NFT)]

        def proj_k_chunk(ft, sc):
            pj = ps_tile([128, 1024], "cd", f"pjk{ft}{sc}")
            for h2 in range(2):
                for mc in range(NFT):
                    nc.tensor.matmul(
                        pj[:, 512 * h2:512 * (h2 + 1)],
                        wk_t[mc][:, 128 * ft:128 * (ft + 1)],
                        xkt[mc][:, 1024 * sc + 512 * h2:1024 * sc + 512 * (h2 + 1)],
                        start=(mc == 0),
                        stop=(mc == NFT - 1),
                    )
            nc.any.tensor_scalar_add(
                k_t[ft][:, 1024 * sc:1024 * (sc + 1)], pj[:], bk_t[ft][:],
            )

        def proj_q_chunk(ft):
            pj = ps_tile([128, 1024], "cd", f"pjq{ft}")
            for h2 in range(2):
                for mc in range(NFT):
                    nc.tensor.matmul(
                        pj[:, 512 * h2:512 * (h2 + 1)],
                        wq_t[mc][:, 128 * ft:128 * (ft + 1)],
                        xqt[mc][:, 512 * h2:512 * (h2 + 1)],
                        start=(mc == 0),
                        stop=(mc == NFT - 1),
                    )
            nc.any.tensor_scalar_add(q_t[ft][:], pj[:], bq_t[ft][:])

        def proj_kq(ft):
            proj_k_chunk(ft, 0)
            proj_q_chunk(ft)
            proj_k_chunk(ft, 1)

        # ---- slot-scheduled emission ----------------------------------
        # PE is the binding engine; emit its work as one interleaved stream:
        #  - S + exp for (pair, kb) runs in slot (pair, kb)
        #  - V projections ride in pair-0 slots (PSUM c/d tags)
        #  - each pair's z-accumulation is deferred while c/d is busy, then
        #    drains two-groups-per-slot once its zA/zB tiles pin c/d
        #  - K/Q projections for pair p+1 slot into the c/d window between
        #    norm(p-1) and z(p) pinning
        proj_k_chunk(0, 0)
        proj_q_chunk(0)

        VW = H * (DH + 1)  # 520: per head 64 value cols + 1 ones col
        v_aug = [v_pool.tile([128, VW], BF16, tag=f"v{kb}", name=f"v{kb}")
                 for kb in range(NKB)]

        def v_group(kb):
            nc.gpsimd.memset(
                v_aug[kb][:].rearrange("p (h c) -> p h c", h=H)[:, :, DH:DH + 1],
                1.0,
            )
            pj = ps_tile([128, 512], "cd", f"pjv{kb}")
            for mc in range(NFT):
                nc.tensor.matmul(
                    pj[:],
                    xvt[mc][:, 128 * kb:128 * (kb + 1)],
                    wv_t[mc][:],
                    start=(mc == 0),
                    stop=(mc == NFT - 1),
                )
            nc.any.tensor_add(
                v_aug[kb][:].rearrange("p (h c) -> p h c", h=H)[:, :, 0:DH],
                pj[:].rearrange("p (h c) -> p h c", h=H),
                bv_bc[:].rearrange("p (h c) -> p h c", h=H),
            )

        z_t = [zt_pool.tile([128, SQ], BF16, tag=f"zt{p}", name=f"zt{p}")
               for p in range(NFT)]
        po_sb = [zt_pool.tile([128, D], BF16, tag=f"po{qb}", name=f"po{qb}")
                 for qb in range(NQB)]
        p_slabs = {}
        z_tiles = {}

        def s_exp(pair, kb):
            sA = ps.tile([128, SQ], F32, tag="a", name=f"sA{pair}_{kb}")
            sB = ps.tile([128, SQ], F32, tag="b", name=f"sB{pair}_{kb}")
            for qc in range(SQ // 512):
                qs = slice(512 * qc, 512 * (qc + 1))
                nc.tensor.matmul(
                    sA[:, qs],
                    k_t[pair][0:DH, 128 * kb:128 * (kb + 1)],
                    q_t[pair][0:DH, qs],
                    start=True, stop=True,
                    tile_position=(0, 0),
                )
                nc.tensor.matmul(
                    sB[:, qs],
                    k_t[pair][DH:128, 128 * kb:128 * (kb + 1)],
                    q_t[pair][DH:128, qs],
                    start=True, stop=True,
                    tile_position=(64, 0),
                )
            pA = p_pool.tile([128, SQ], BF16, tag="p", name=f"pA{pair}_{kb}")
            nc.scalar.activation(pA[:], sA[:], Exp, scale=0.125)
            pB = p_pool.tile([128, SQ], BF16, tag="p", name=f"pB{pair}_{kb}")
            nc.scalar.activation(pB[:], sB[:], Exp, scale=0.125)
            p_slabs[(pair, kb)] = (pA, pB)

        def z_alloc(pair):
            zA = ps.tile([DH + 1, SQ], F32, tag="c", name=f"zA{pair}",
                         padded_shape=[128, SQ])
            zB = ps.tile([DH + 1, SQ], F32, tag="d", name=f"zB{pair}",
                         padded_shape=[128, SQ])
            z_tiles[pair] = (zA, zB)

        def z_group(pair, kb):
            zA, zB = z_tiles[pair]
            pA, pB = p_slabs.pop((pair, kb))
            hA, hB = 2 * pair, 2 * pair + 1
            for qc in range(SQ // 512):
                qs = slice(512 * qc, 512 * (qc + 1))
                nc.tensor.matmul(
                    zA[:, qs],
                    v_aug[kb][:, 65 * hA:65 * hA + 65],
                    pA[:, qs],
                    start=(kb == 0), stop=(kb == NKB - 1),
                    skip_group_check=True,
                )
                nc.tensor.matmul(
                    zB[:, qs],
                    v_aug[kb][:, 65 * hB:65 * hB + 65],
                    pB[:, qs],
                    start=(kb == 0), stop=(kb == NKB - 1),
                    skip_group_check=True,
                )

        def norm(pair):
            # Head A's raw z is drained to SBUF immediately (3 DVE reads
            # free PSUM tag c ~4us earlier for the next projections) and
            # normalized in place later; head B keeps the direct PSUM-read
            # multiply (all DVE input operands must stay at partition base
            # 0, so an in-place z_t[64:128] multiply is not expressible).
            zA, zB = z_tiles.pop(pair)
            rowcA = nrm_pool.tile([1, SQ], F32, tag="rowc")
            nc.vector.tensor_copy(rowcA[:], zA[DH:DH + 1, :])
            nc.vector.tensor_copy(z_t[pair][0:DH, :], zA[0:DH, :])
            rowcB = nrm_pool.tile([1, SQ], F32, tag="rowc")
            nc.vector.tensor_copy(rowcB[:], zB[DH:DH + 1, :])
            for rowc, half, src in ((rowcA, 0, None), (rowcB, 1, zB)):
                recip = nrm_pool.tile([1, SQ], F32, tag="recip")
                nc.vector.reciprocal_approx_fast(recip[:], rowc[:])
                rbc = nrm_pool.tile([DH, SQ], F32, tag="rbc")
                nc.gpsimd.partition_broadcast(rbc[:], recip[:], channels=DH)
                nc.vector.tensor_mul(
                    z_t[pair][64 * half:64 * half + 64, :],
                    z_t[pair][0:DH, :] if src is None else src[0:DH, :],
                    rbc[:]
                )

        # slot schedule: slot (p, kb) -> extra emissions after S+exp
        feeder = [lambda: proj_k_chunk(0, 1),
                  lambda: proj_k_chunk(1, 0),
                  lambda: proj_q_chunk(1),
                  lambda: proj_k_chunk(1, 1)]
        feeder += [(lambda k: (lambda: v_group(k)))(kb) for kb in range(NKB)]
        fi = [0]

        def feed(n):
            for _ in range(n):
                if fi[0] < len(feeder):
                    feeder[fi[0]]()
                    fi[0] += 1

        for pair in range(NFT):
            for kb in range(NKB):
                s_exp(pair, kb)
                if pair == 0:
                    feed(2 if kb < 4 else 1)
                    if kb == NKB - 1:
                        feed(len(feeder))
                elif pair == 1:
                    if kb == 0:
                        z_alloc(0)
                    if kb < 8:
                        z_group(0, 2 * kb)
                        z_group(0, 2 * kb + 1)
                    elif kb == 8:
                        norm(0)
                    elif kb == 9:
                        proj_k_chunk(2, 0)
                    elif kb == 10:
                        proj_q_chunk(2)
                    elif kb == 11:
                        proj_k_chunk(2, 1)
                    elif kb == 12:
                        z_alloc(1)
                    if kb >= 12:
                        z_group(1, 2 * (kb - 12))
                        z_group(1, 2 * (kb - 12) + 1)
                elif pair == 2:
                    if kb < 4:
                        z_group(1, 8 + 2 * kb)
                        z_group(1, 8 + 2 * kb + 1)
                    elif kb == 4:
                        norm(1)
                    elif kb == 5:
                        proj_k_chunk(3, 0)
                    elif kb == 6:
                        proj_q_chunk(3)
                    elif kb == 7:
                        proj_k_chunk(3, 1)
                    elif kb == 8:
                        z_alloc(2)
                    if kb >= 8:
                        z_group(2, 2 * (kb - 8))
                        z_group(2, 2 * (kb - 8) + 1)
                else:
                    if kb == 0:
                        norm(2)
                    elif kb == 1:
                        z_alloc(3)
                    if kb >= 1:
                        z_group(3, kb - 1)
            if pair == NFT - 1:
                z_group(3, 15)
                # norm(3) owns DVE/GpSimd; out-proj pass 1 (pairs 0-2 plus
                # the bias as a rank-1 matmul) runs on the PE with the freed
                # a/b banks and drains via the now-idle ScalarE
                norm(3)
                for qb in range(NQB):
                    po = ps_tile([128, D], "ab", f"po1{qb}")
                    for p4 in range(3):
                        nc.tensor.matmul(
                            po[:],
                            z_t[p4][:, 128 * qb:128 * (qb + 1)],
                            wo_t[p4][:],
                            start=(p4 == 0),
                            stop=False,
                        )
                    nc.tensor.matmul(po[:], ones_row[:], bo_rb[:],
                                     start=False, stop=True)
                    nc.scalar.activation(po_sb[qb][:], po[:],
                                         mybir.ActivationFunctionType.Copy)

        # ---- output projection pass 2: add pair 3, DMA out (scalar queue
        # is busy with the pass-1 drains, so DMA via sync/gpsimd) ----
        out_engs = [nc.sync, nc.gpsimd]
        for qb in range(NQB):
            po = ps_tile([128, D], "ab", f"po2{qb}")
            nc.tensor.matmul(
                po[:],
                z_t[3][:, 128 * qb:128 * (qb + 1)],
                wo_t[3][:],
                start=True, stop=True,
            )
            ot = out_pool.tile([128, D], F32, tag="ot")
            nc.any.tensor_add(ot[:], po[:], po_sb[qb][:])
            out_engs[qb % 2].dma_start(out[128 * qb:128 * (qb + 1), :], ot[:])

    nc.compile()
    return nc


_NC = None
LAST_RESULTS = None


def _get_nc():
    global _NC
    if _NC is None:
        _NC = build_program(dbg=bool(int(os.environ.get("KERNEL_DEBUG", "0"))))
    return _NC


def _bf(x):
    return np.ascontiguousarray(np.asarray(x).astype(ml_dtypes.bfloat16))


def kernel(Q, K, V, Wq, bq, Wk, bk, Wv, bv, Wo, bo):
    global LAST_RESULTS
    nc = _get_nc()
    Qb, Kb, Vb = _bf(Q), _bf(K), _bf(V)
    shared = {
        "Wq": _bf(Wq),
        "Wk": _bf(Wk),
        "Wv": _bf(Wv),
        "Wo": _bf(Wo),
        "bq": np.ascontiguousarray(np.asarray(bq, np.float32).reshape(D, 1)),
        "bk": np.ascontiguousarray(np.asarray(bk, np.float32).reshape(D, 1)),
        "bv": np.ascontiguousarray(np.asarray(bv, np.float32).reshape(1, D)),
        "bo": np.ascontiguousarray(np.asarray(bo, np.float32).reshape(1, D)),
    }
    kt = [np.ascontiguousarray(Kb[b].T) for b in range(B)]
    vt = [np.ascontiguousarray(Vb[b].T) for b in range(B)]
    in_maps = []
    for c in range(N_CORES):
        b, qh = c // 2, c % 2
        in_maps.append({
            "XQT": np.ascontiguousarray(Qb[b, SQ * qh:SQ * (qh + 1)].T),
            "XKT": kt[b],
            "XVT": vt[b],
            **shared,
        })
    trace = bool(int(os.environ.get("KERNEL_TRACE", "0")))
    res = bass_utils.run_bass_kernel_spmd(
        nc, in_maps, core_ids=list(range(N_CORES)), trace=trace,
    )
    LAST_RESULTS = res
    out = np.empty((B, S, D), dtype=np.float32)
    for c in range(N_CORES):
        b, qh = c // 2, c % 2
        out[b, SQ * qh:SQ * (qh + 1)] = res.results[c]["OUT"]
    return out


# revision 91
# speedup vs baseline: 1.0158x; 1.0088x over previous
"""Trainium2 Bass kernel for nn_MultiHeadAttention (B=4, S=2048, D=512, H=8, DH=64).

Sharding: 8 cores = 4 batches x 2 query-halves. Each core computes full
attention for all 8 heads over its 1024 query rows (K/V projections are
duplicated within a batch pair). The output is a pure concatenation.

Per-core pipeline (bf16 datapath, fp32 PSUM accumulation):
  1. Inputs/weights are pre-cast to bf16 AND pre-transposed on the host;
     X^T (feature-major) loads are plain contiguous DMAs spread over the
     sync + scalar queues (on-device DMA_TRANSPOSE costs ~2.4us of
     descriptor-generation per chunk and serialized the whole prologue).
  2. Dummy matmuls on a memset tile warm the PE clock (HAM) to K=8/8
     during the DMA prologue.
  3. Project: Q^T, K^T feature-major ([512, S]); V natural ([S, 512]) with
     an extra all-ones column appended per head (65-col layout).
  4. Attention per head, per 128-row k-block:
       S^T[k, q] = K^T_h(stationary) @ Q^T_h   (contraction = d_head 64)
       P^T = exp(S^T / 8)                       (ScalarE, PSUM -> SBUF bf16)
       z^T[65, q] += [V_h | 1](stationary) @ P^T  (row 64 = softmax denom)
  5. Normalize: reciprocal of row 64, broadcast, multiply -> Z^T.
  6. Output projection from Z^T + bias in fp32, DMA out.
"""

import os
import sys

import numpy as np

sys.path.insert(0, "/opt/trn_rl_repo")

import ml_dtypes
import concourse.bacc as bacc
import concourse.bass as bass
import concourse.mybir as mybir
import concourse.tile as tile
from concourse import bass_utils

F32 = mybir.dt.float32
BF16 = mybir.dt.bfloat16

B, S, D, H, DH = 4, 2048, 512, 8, 64
SQ = S // 2          # query rows per core
NKB = S // 128       # 16 k-blocks
NFT = D // 128       # 4 feature tiles
NQB = SQ // 128      # 8 query blocks
N_CORES = 8

Exp = mybir.ActivationFunctionType.Exp
Identity = mybir.ActivationFunctionType.Identity


def build_program(dbg=False):
    nc = bacc.Bacc("TRN2", target_bir_lowering=False, debug=False)

    xq = nc.dram_tensor("XQT", [D, SQ], BF16, kind="ExternalInput").ap()
    xk = nc.dram_tensor("XKT", [D, S], BF16, kind="ExternalInput").ap()
    xv = nc.dram_tensor("XVT", [D, S], BF16, kind="ExternalInput").ap()
    wq = nc.dram_tensor("Wq", [D, D], BF16, kind="ExternalInput").ap()
    wk = nc.dram_tensor("Wk", [D, D], BF16, kind="ExternalInput").ap()
    wv = nc.dram_tensor("Wv", [D, D], BF16, kind="ExternalInput").ap()
    wo = nc.dram_tensor("Wo", [D, D], BF16, kind="ExternalInput").ap()
    bq = nc.dram_tensor("bq", [D, 1], F32, kind="ExternalInput").ap()
    bk = nc.dram_tensor("bk", [D, 1], F32, kind="ExternalInput").ap()
    bv = nc.dram_tensor("bv", [1, D], F32, kind="ExternalInput").ap()
    bo = nc.dram_tensor("bo", [1, D], F32, kind="ExternalInput").ap()
    out = nc.dram_tensor("OUT", [SQ, D], F32, kind="ExternalOutput").ap()

    from contextlib import ExitStack

    with tile.TileContext(nc) as tc, ExitStack() as ctx:
        const = ctx.enter_context(tc.tile_pool(name="const", bufs=1))
        xt_pool = ctx.enter_context(tc.tile_pool(name="xt", bufs=1))
        w_pool = ctx.enter_context(tc.tile_pool(name="w", bufs=1))
        kt_pool = ctx.enter_context(tc.tile_pool(name="kt", bufs=1))
        qt_pool = ctx.enter_context(tc.tile_pool(name="qt", bufs=1))
        v_pool = ctx.enter_context(tc.tile_pool(name="v", bufs=1))
        # 27 exp-slab buffers: each pair produces 32 slabs before the next
        # pair's z-drain consumes them, so every buffer beyond 23 directly
        # shortens the exp WAR stall at pair boundaries
        p_pool = ctx.enter_context(tc.tile_pool(name="p", bufs=33))
        zt_pool = ctx.enter_context(tc.tile_pool(name="zt", bufs=1))
        nrm_pool = ctx.enter_context(tc.tile_pool(name="nrm", bufs=1))
        out_pool = ctx.enter_context(tc.tile_pool(name="outp", bufs=3))

        # One PSUM pool, 4 tags x 2 banks = all 8 banks.  Prefix (projection)
        # and epilogue tiles rotate through the same tags that attention uses
        # for sA/sB/zA/zB.
        ps = ctx.enter_context(tc.tile_pool(name="ps", bufs=1, space="PSUM"))
        ps_ctr = [0, 0]

        def ps_tile(shape, tags, name):
            i = 0 if tags == "ab" else 1
            tag = ("a", "b", "c", "d")[2 * i + ps_ctr[i] % 2]
            ps_ctr[i] += 1
            return ps.tile(shape, F32, tag=tag, name=name, padded_shape=[128, SQ])

        # warm the ScalarE Exp table first thing (the first ACTIVATE
        # otherwise pays the ~2.7us ACT_TABLE_LOAD inline)
        warm = nrm_pool.tile([1, 8], F32, tag="warm")
        nc.gpsimd.memset(warm[:], 0.0)
        warm2 = nrm_pool.tile([1, 8], F32, tag="warm2")
        nc.scalar.activation(warm2[:], warm[:], Exp, scale=0.125)

        # PE warm-up: dummy matmuls on a memset tile so HAM reaches K=8/8
        # before the first real matmul (the DMA prologue would otherwise
        # leave the PE at half clock for its first ~3.4us of work)
        scratch = const.tile([128, 256], BF16, tag="scratch")
        nc.gpsimd.memset(scratch[:], 0.0)
        dummy_ps = ps.tile([128, 256], F32, tag="a", name="dummy",
                           padded_shape=[128, SQ])
        for _ in range(48):
            nc.tensor.matmul(dummy_ps[:], scratch[:, 0:128], scratch[:],
                             start=True, stop=True)

        # ---- weights: one DMA per tensor, sliced into 4 contraction chunks ----
        def load_w(wdram, name, eng):
            big = w_pool.tile([128, NFT * D], BF16, tag=f"w{name}", name=f"w{name}")
            eng.dma_start(
                big[:].rearrange("p (g c) -> p g c", g=NFT),
                wdram.rearrange("(g p) c -> p g c", p=128),
            )
            return [big[:, D * mc:D * (mc + 1)] for mc in range(NFT)]

        # ---- X^T loads: plain contiguous DMAs (host pre-transposed X) ----
        def load_xt(xdram, nrows, name, engs):
            tiles = []
            for ft in range(NFT):
                t = xt_pool.tile([128, nrows], BF16, tag=f"xt{name}{ft}",
                                 name=f"xt{name}{ft}", padded_shape=[128, S])
                engs[ft % len(engs)].dma_start(
                    t[:], xdram[128 * ft:128 * (ft + 1), :]
                )
                tiles.append(t)
            return tiles

        # ---- DMA loads: priority order.  The first exp needs wk + K-cols
        # 0-1023 + wq + all of Q; those go first, with the X chunks split
        # across the scalar/gpsimd queues so transfers overlap the weight
        # loads running on sync. ----
        def xt_tiles(name, nrows):
            # padded to the actual row count (padding xq to [128,S] wasted
            # 8KB/partition that the p pool needs for exp slabs)
            return [xt_pool.tile([128, nrows], BF16, tag=f"xt{name}{ft}",
                                 name=f"xt{name}{ft}", padded_shape=[128, nrows])
                    for ft in range(NFT)]

        xkt = xt_tiles("k", S)
        xqt = xt_tiles("q", SQ)
        xvt = xt_tiles("v", S)

        # balance the 3MB of first-exp-critical bytes across the two HWDGE
        # queues (sync ~1.5MB, scalar ~1.5MB at ~130GB/s each)
        wk_t = load_w(wk, "k", nc.sync)
        wq_t = load_w(wq, "q", nc.scalar)
        for ft in range(NFT):           # K-blocks 0-7 (needed by s_exp(0,0))
            (nc.sync if ft < 2 else nc.scalar).dma_start(
                xkt[ft][:, 0:1024], xk[128 * ft:128 * (ft + 1), 0:1024])
        for ft in range(NFT):           # all of Q
            (nc.sync if ft < 2 else nc.scalar).dma_start(
                xqt[ft][:], xq[128 * ft:128 * (ft + 1), :])
        for ft in range(NFT):           # K-blocks 8-15 (needed from slot 8)
            (nc.sync if ft < 2 else nc.scalar).dma_start(
                xkt[ft][:, 1024:2048], xk[128 * ft:128 * (ft + 1), 1024:2048])
        bq_all = const.tile([128, NFT], F32, tag="bqa")
        nc.sync.dma_start(
            bq_all[:].rearrange("p (g o) -> p g o", g=NFT),
            bq.rearrange("(g p) o -> p g o", p=128),
        )
        bk_all = const.tile([128, NFT], F32, tag="bka")
        nc.sync.dma_start(
            bk_all[:].rearrange("p (g o) -> p g o", g=NFT),
            bk.rearrange("(g p) o -> p g o", p=128),
        )
        bq_t = [bq_all[:, ft:ft + 1] for ft in range(NFT)]
        bk_t = [bk_all[:, ft:ft + 1] for ft in range(NFT)]
        bv_row = const.tile([1, D], F32, tag="bvr")
        nc.sync.dma_start(bv_row[:], bv[:])
        bv_bc = const.tile([128, D], F32, tag="bvb")
        nc.gpsimd.partition_broadcast(bv_bc[:], bv_row[:], channels=128)
        bo_row = const.tile([1, D], F32, tag="bor")
        nc.sync.dma_start(bo_row[:], bo[:])
        bo_rb = const.tile([1, D], BF16, tag="borb")
        nc.vector.tensor_copy(bo_rb[:], bo_row[:])
        ones_row = const.tile([1, 128], BF16, tag="ones")
        nc.gpsimd.memset(ones_row[:], 1.0)

        wv_t = load_w(wv, "v", nc.sync)
        for hh in range(2):             # V halves, k-blocks 0-7 first
            for ft in range(NFT):
                (nc.sync if ft % 2 else nc.scalar).dma_start(
                    xvt[ft][:, 1024 * hh:1024 * (hh + 1)],
                    xv[128 * ft:128 * (ft + 1), 1024 * hh:1024 * (hh + 1)])
        wo_t = load_w(wo, "o", nc.sync)

        k_t = [kt_pool.tile([128, S], BF16, tag=f"kt{ft}", name=f"kt{ft}")
               for ft in range(NFT)]
        q_t = [qt_pool.tile([128, SQ], BF16, tag=f"qt{ft}", name=f"qt{ft}")
               for ft in range(NFT)]

        def proj_k_chunk(ft, sc):
            pj = ps_tile([128, 1024], "cd", f"pjk{ft}{sc}")
            for h2 in range(2):
                for mc in range(NFT):
                    nc.tensor.matmul(
                        pj[:, 512 * h2:512 * (h2 + 1)],
                        wk_t[mc][:, 128 * ft:128 * (ft + 1)],
                        xkt[mc][:, 1024 * sc + 512 * h2:1024 * sc + 512 * (h2 + 1)],
                        start=(mc == 0),
                        stop=(mc == NFT - 1),
                    )
            nc.any.tensor_scalar_add(
                k_t[ft][:, 1024 * sc:1024 * (sc + 1)], pj[:], bk_t[ft][:],
            )

        def proj_q_chunk(ft):
            pj = ps_tile([128, 1024], "cd", f"pjq{ft}")
            for h2 in range(2):
                for mc in range(NFT):
                    nc.tensor.matmul(
                        pj[:, 512 * h2:512 * (h2 + 1)],
                        wq_t[mc][:, 128 * ft:128 * (ft + 1)],
                        xqt[mc][:, 512 * h2:512 * (h2 + 1)],
                        start=(mc == 0),
                        stop=(mc == NFT - 1),
                    )
            nc.any.tensor_scalar_add(q_t[ft][:], pj[:], bq_t[ft][:])

        def proj_kq(ft):
            proj_k_chunk(ft, 0)
            proj_q_chunk(ft)
            proj_k_chunk(ft, 1)

        # ---- slot-scheduled emission ----------------------------------
        # PE is the binding engine; emit its work as one interleaved stream:
        #  - S + exp for (pair, kb) runs in slot (pair, kb)
        #  - V projections ride in pair-0 slots (PSUM c/d tags)
        #  - each pair's z-accumulation is deferred while c/d is busy, then
        #    drains two-groups-per-slot once its zA/zB tiles pin c/d
        #  - K/Q projections for pair p+1 slot into the c/d window between
        #    norm(p-1) and z(p) pinning
        proj_k_chunk(0, 0)
        proj_q_chunk(0)

        VW = H * (DH + 1)  # 520: per head 64 value cols + 1 ones col
        v_aug = [v_pool.tile([128, VW], BF16, tag=f"v{kb}", name=f"v{kb}")
                 for kb in range(NKB)]

        def v_group(kb):
            nc.gpsimd.memset(
                v_aug[kb][:].rearrange("p (h c) -> p h c", h=H)[:, :, DH:DH + 1],
                1.0,
            )
            pj = ps_tile([128, 512], "cd", f"pjv{kb}")
            for mc in range(NFT):
                nc.tensor.matmul(
                    pj[:],
                    xvt[mc][:, 128 * kb:128 * (kb + 1)],
                    wv_t[mc][:],
                    start=(mc == 0),
                    stop=(mc == NFT - 1),
                )
            nc.any.tensor_add(
                v_aug[kb][:].rearrange("p (h c) -> p h c", h=H)[:, :, 0:DH],
                pj[:].rearrange("p (h c) -> p h c", h=H),
                bv_bc[:].rearrange("p (h c) -> p h c", h=H),
            )

        z_t = [zt_pool.tile([128, SQ], BF16, tag=f"zt{p}", name=f"zt{p}")
               for p in range(NFT)]
        po_sb = [zt_pool.tile([128, D], BF16, tag=f"po{qb}", name=f"po{qb}")
                 for qb in range(NQB)]
        p_slabs = {}
        z_tiles = {}

        def s_exp(pair, kb):
            sA = ps.tile([128, SQ], F32, tag="a", name=f"sA{pair}_{kb}")
            sB = ps.tile([128, SQ], F32, tag="b", name=f"sB{pair}_{kb}")
            for qc in range(SQ // 512):
                qs = slice(512 * qc, 512 * (qc + 1))
                nc.tensor.matmul(
                    sA[:, qs],
                    k_t[pair][0:DH, 128 * kb:128 * (kb + 1)],
                    q_t[pair][0:DH, qs],
                    start=True, stop=True,
                    tile_position=(0, 0),
                )
                nc.tensor.matmul(
                    sB[:, qs],
                    k_t[pair][DH:128, 128 * kb:128 * (kb + 1)],
                    q_t[pair][DH:128, qs],
                    start=True, stop=True,
                    tile_position=(64, 0),
                )
            pA = p_pool.tile([128, SQ], BF16, tag="p", name=f"pA{pair}_{kb}")
            nc.scalar.activation(pA[:], sA[:], Exp, scale=0.125)
            pB = p_pool.tile([128, SQ], BF16, tag="p", name=f"pB{pair}_{kb}")
            nc.scalar.activation(pB[:], sB[:], Exp, scale=0.125)
            p_slabs[(pair, kb)] = (pA, pB)

        def z_alloc(pair):
            zA = ps.tile([DH + 1, SQ], F32, tag="c", name=f"zA{pair}",
                         padded_shape=[128, SQ])
            zB = ps.tile([DH + 1, SQ], F32, tag="d", name=f"zB{pair}",
                         padded_shape=[128, SQ])
            z_tiles[pair] = (zA, zB)

        def z_group(pair, kb):
            zA, zB = z_tiles[pair]
            pA, pB = p_slabs.pop((pair, kb))
            hA, hB = 2 * pair, 2 * pair + 1
            for qc in range(SQ // 512):
                qs = slice(512 * qc, 512 * (qc + 1))
                nc.tensor.matmul(
                    zA[:, qs],
                    v_aug[kb][:, 65 * hA:65 * hA + 65],
                    pA[:, qs],
                    start=(kb == 0), stop=(kb == NKB - 1),
                    skip_group_check=True,
                )
                nc.tensor.matmul(
                    zB[:, qs],
                    v_aug[kb][:, 65 * hB:65 * hB + 65],
                    pB[:, qs],
                    start=(kb == 0), stop=(kb == NKB - 1),
                    skip_group_check=True,
                )

        def norm(pair):
            # Head A's raw z is drained to SBUF immediately (3 DVE reads
            # free PSUM tag c ~4us earlier for the next projections) and
            # normalized in place later; head B keeps the direct PSUM-read
            # multiply (all DVE input operands must stay at partition base
            # 0, so an in-place z_t[64:128] multiply is not expressible).
            zA, zB = z_tiles.pop(pair)
            rowcA = nrm_pool.tile([1, SQ], F32, tag="rowc")
            nc.vector.tensor_copy(rowcA[:], zA[DH:DH + 1, :])
            nc.vector.tensor_copy(z_t[pair][0:DH, :], zA[0:DH, :])
            rowcB = nrm_pool.tile([1, SQ], F32, tag="rowc")
            nc.vector.tensor_copy(rowcB[:], zB[DH:DH + 1, :])
            for rowc, half, src in ((rowcA, 0, None), (rowcB, 1, zB)):
                recip = nrm_pool.tile([1, SQ], F32, tag="recip")
                nc.vector.reciprocal_approx_fast(recip[:], rowc[:])
                rbc = nrm_pool.tile([DH, SQ], F32, tag="rbc")
                nc.gpsimd.partition_broadcast(rbc[:], recip[:], channels=DH)
                nc.vector.tensor_mul(
                    z_t[pair][64 * half:64 * half + 64, :],
                    z_t[pair][0:DH, :] if src is None else src[0:DH, :],
                    rbc[:]
                )

        # slot schedule: slot (p, kb) -> extra emissions after S+exp
        feeder = [lambda: proj_k_chunk(0, 1),
                  lambda: proj_k_chunk(1, 0),
                  lambda: proj_q_chunk(1),
                  lambda: proj_k_chunk(1, 1)]
        feeder += [(lambda k: (lambda: v_group(k)))(kb) for kb in range(NKB)]
        fi = [0]

        def feed(n):
            for _ in range(n):
                if fi[0] < len(feeder):
                    feeder[fi[0]]()
                    fi[0] += 1

        for pair in range(NFT):
            for kb in range(NKB):
                s_exp(pair, kb)
                if pair == 0:
                    feed(2 if kb < 4 else 1)
                    if kb == NKB - 1:
                        feed(len(feeder))
                elif pair == 1:
                    if kb == 0:
                        z_alloc(0)
                    if kb < 8:
                        z_group(0, 2 * kb)
                        z_group(0, 2 * kb + 1)
                    elif kb == 8:
                        norm(0)
                    elif kb == 9:
                        proj_k_chunk(2, 0)
                    elif kb == 10:
                        proj_q_chunk(2)
                    elif kb == 11:
                        proj_k_chunk(2, 1)
                    elif kb == 12:
                        z_alloc(1)
                    if kb >= 12:
                        z_group(1, 2 * (kb - 12))
                        z_group(1, 2 * (kb - 12) + 1)
                elif pair == 2:
                    if kb < 4:
                        z_group(1, 8 + 2 * kb)
                        z_group(1, 8 + 2 * kb + 1)
                    elif kb == 4:
                        norm(1)
                    elif kb == 5:
                        proj_k_chunk(3, 0)
                    elif kb == 6:
                        proj_q_chunk(3)
                    elif kb == 7:
                        proj_k_chunk(3, 1)
                    elif kb == 8:
                        z_alloc(2)
                    if kb >= 8:
                        z_group(2, 2 * (kb - 8))
                        z_group(2, 2 * (kb - 8) + 1)
                else:
                    if kb == 0:
                        norm(2)
                    elif kb == 1:
                        z_alloc(3)
                    if kb >= 1:
                        z_group(3, kb - 1)
            if pair == NFT - 1:
                z_group(3, 15)
                # norm(3) owns DVE/GpSimd; out-proj pass 1 (pairs 0-2 plus
                # the bias as a rank-1 matmul) runs on the PE with the freed
                # a/b banks and drains via the now-idle ScalarE
                norm(3)
                for qb in range(NQB):
                    po = ps_tile([128, D], "ab", f"po1{qb}")
                    for p4 in range(3):
                        nc.tensor.matmul(
                            po[:],
                            z_t[p4][:, 128 * qb:128 * (qb + 1)],
                            wo_t[p4][:],
                            start=(p4 == 0),
                            stop=False,
                        )
                    nc.tensor.matmul(po[:], ones_row[:], bo_rb[:],
                                     start=False, stop=True)
                    nc.scalar.activation(po_sb[qb][:], po[:],
                                         mybir.ActivationFunctionType.Copy)

        # ---- output projection pass 2: add pair 3, DMA out (scalar queue
        # is busy with the pass-1 drains, so DMA via sync/gpsimd) ----
        out_engs = [nc.sync, nc.gpsimd]
        for qb in range(NQB):
            po = ps_tile([128, D], "ab", f"po2{qb}")
            nc.tensor.matmul(
                po[:],
                z_t[3][:, 128 * qb:128 * (qb + 1)],
                wo_t[3][:],
                start=True, stop=True,
            )
            ot = out_pool.tile([128, D], F32, tag="ot")
            nc.any.tensor_add(ot[:], po[:], po_sb[qb][:])
            out_engs[qb % 2].dma_start(out[128 * qb:128 * (qb + 1), :], ot[:])

    nc.compile()
    return nc


_NC = None
LAST_RESULTS = None


def _get_nc():
    global _NC
    if _NC is None:
        _NC = build_program(dbg=bool(int(os.environ.get("KERNEL_DEBUG", "0"))))
    return _NC


def _bf(x):
    return np.ascontiguousarray(np.asarray(x).astype(ml_dtypes.bfloat16))


def kernel(Q, K, V, Wq, bq, Wk, bk, Wv, bv, Wo, bo):
    global LAST_RESULTS
    nc = _get_nc()
    Qb, Kb, Vb = _bf(Q), _bf(K), _bf(V)
    shared = {
        "Wq": _bf(Wq),
        "Wk": _bf(Wk),
        "Wv": _bf(Wv),
        "Wo": _bf(Wo),
        "bq": np.ascontiguousarray(np.asarray(bq, np.float32).reshape(D, 1)),
        "bk": np.ascontiguousarray(np.asarray(bk, np.float32).reshape(D, 1)),
        "bv": np.ascontiguousarray(np.asarray(bv, np.float32).reshape(1, D)),
        "bo": np.ascontiguousarray(np.asarray(bo, np.float32).reshape(1, D)),
    }
    kt = [np.ascontiguousarray(Kb[b].T) for b in range(B)]
    vt = [np.ascontiguousarray(Vb[b].T) for b in range(B)]
    in_maps = []
    for c in range(N_CORES):
        b, qh = c // 2, c % 2
        in_maps.append({
            "XQT": np.ascontiguousarray(Qb[b, SQ * qh:SQ * (qh + 1)].T),
            "XKT": kt[b],
            "XVT": vt[b],
            **shared,
        })
    trace = bool(int(os.environ.get("KERNEL_TRACE", "0")))
    res = bass_utils.run_bass_kernel_spmd(
        nc, in_maps, core_ids=list(range(N_CORES)), trace=trace,
    )
    LAST_RESULTS = res
    out = np.empty((B, S, D), dtype=np.float32)
    for c in range(N_CORES):
        b, qh = c // 2, c % 2
        out[b, SQ * qh:SQ * (qh + 1)] = res.results[c]["OUT"]
    return out
